# revision 7
# baseline (speedup 1.0000x reference)
"""Cumulative-probability head on 8 Trainium2 NeuronCores.

out[b, j] = sum_{i<=j} relu(x @ W_h^T + b_h)[b, i] + (x @ W_base^T + b_base)[b]

Data-parallel: x is sharded along batch (1024 rows per core); the small
weights are replicated. Per core:
  1. PE-transposes of x tiles (fp32r transpose mode, 4 per PSUM bank)
  2. haz = xT.T @ WT_aug in float32r (FP22 multiplies, fp32 PSUM accum).
     WT_aug is [2049, 513]: hazard cols 0..511, base col 512; row 2048 is
     the bias row, folded in with a K=1 ones-row matmul.
  3. ReLU on ScalarE (hazard cols only; base col copied unactivated)
  4. inclusive cumsum along T via DVE tensor_tensor_scan with the base
     hazard as the per-partition initial state.
"""

import numpy as np

import concourse.bass as bass
import concourse.tile as tile
from concourse import bacc, mybir
from concourse.bass_utils import run_bass_kernel_spmd

B, D, T = 8192, 2048, 512
NCORES = 8
BLOC = B // NCORES            # 1024 rows per core
NB = BLOC // 128              # 8 batch tiles per core
NK = D // 128                 # 16 contraction chunks
NG = NK // 4                  # 4 transpose groups (4 chunks per PSUM bank)
TP = 516                      # padded output width (512 hazard + base + 3 junk)
NA = 258                      # output cols 0..257 in PSUM bank A
NBK = TP - NA                 # cols 258..515 in bank B (col 512 = base)
BOFF = T - NA                 # offset of the base col inside bank B (254)

F32 = mybir.dt.float32
F32R = mybir.dt.float32r


def _build_program():
    nc = bacc.Bacc("TRN2", target_bir_lowering=False, debug=False)

    x_d = nc.dram_tensor("x", [BLOC, D], F32R, kind="ExternalInput")
    wt_d = nc.dram_tensor("wt", [D + 1, TP], F32R, kind="ExternalInput")
    ident_d = nc.dram_tensor("ident", [128, 128], F32R, kind="ExternalInput")
    ones_d = nc.dram_tensor("ones", [1, 128], F32R, kind="ExternalInput")
    out_d = nc.dram_tensor("out", [BLOC, T], F32, kind="ExternalOutput")

    with tile.TileContext(nc) as tc:
        with (
            tc.tile_pool(name="consts", bufs=1) as consts,
            tc.tile_pool(name="wt", bufs=1) as wtp,
            tc.tile_pool(name="xnat", bufs=3) as xnat,
            tc.tile_pool(name="xt", bufs=2) as xtp,
            tc.tile_pool(name="haz", bufs=2) as hazp,
            tc.tile_pool(name="outp", bufs=3) as outp,
            tc.tile_pool(name="ps_xt", bufs=2, space="PSUM") as ps_xt,
            tc.tile_pool(name="ps_mm", bufs=2, space="PSUM") as ps_mm,
        ):
            ident = consts.tile([128, 128], F32R)
            nc.sync.dma_start(out=ident, in_=ident_d[:])
            ones_r = consts.tile([1, 128], F32R)
            nc.sync.dma_start(out=ones_r, in_=ones_d[:])
            zeros = consts.tile([128, T], F32)
            nc.vector.memset(zeros, 0.0)

            wt_tiles = []
            for k in range(NK):
                w = wtp.tile([128, TP], F32R, tag=f"wt{k}")
                nc.sync.dma_start(out=w, in_=wt_d[128 * k : 128 * (k + 1), :])
                wt_tiles.append(w)
            wbias = wtp.tile([1, TP], F32R, tag="wbias")
            nc.sync.dma_start(out=wbias, in_=wt_d[D : D + 1, :])

            for b in range(NB):
                xn = xnat.tile([128, D], F32R)
                nc.sync.dma_start(out=xn, in_=x_d[128 * b : 128 * (b + 1), :])

                # Transpose 16 [128,128] chunks of x, 4 per PSUM bank.
                xt_sb = []
                for g in range(NG):
                    pxt = ps_xt.tile([128, 4, 128], F32R, tag="pxt")
                    for j in range(4):
                        k = 4 * g + j
                        nc.tensor.transpose(
                            pxt[:, j, :],
                            xn[:, 128 * k : 128 * (k + 1)],
                            ident,
                        )
                    xts = xtp.tile([128, 4, 128], F32R, tag=f"xt{g}")
                    nc.vector.tensor_copy(out=xts, in_=pxt)
                    xt_sb.append(xts)

                pA = ps_mm.tile([128, NA], F32, tag="pA")
                pB = ps_mm.tile([128, NBK], F32, tag="pB")
                for k in range(NK):
                    xt_ap = xt_sb[k // 4][:, k % 4, :]
                    w = wt_tiles[k]
                    nc.tensor.matmul(
                        pA[:],
                        xt_ap,
                        w[:, 0:NA],
                        start=(k == 0),
                        stop=False,
                    )
                    nc.tensor.matmul(
                        pB[:],
                        xt_ap,
                        w[:, NA:TP],
                        start=(k == 0),
                        stop=False,
                    )
                # Bias row: K=1 matmul against a ones row.
                nc.tensor.matmul(
                    pA[:],
                    ones_r,
                    wbias[:, 0:NA],
                    start=False,
                    stop=True,
                )
                nc.tensor.matmul(
                    pB[:],
                    ones_r,
                    wbias[:, NA:TP],
                    start=False,
                    stop=True,
                )

                haz = hazp.tile([128, T], F32, tag="haz")
                base = hazp.tile([128, 1], F32, tag="base")
                nc.scalar.activation(
                    out=haz[:, 0:NA],
                    in_=pA[:],
                    func=mybir.ActivationFunctionType.Relu,
                )
                nc.scalar.activation(
                    out=haz[:, NA:T],
                    in_=pB[:, 0:BOFF],
                    func=mybir.ActivationFunctionType.Relu,
                )
                nc.scalar.copy(out=base, in_=pB[:, BOFF : BOFF + 1])

                cum = outp.tile([128, T], F32)
                nc.vector.tensor_tensor_scan(
                    out=cum,
                    data0=haz,
                    data1=zeros,
                    initial=base,
                    op0=mybir.AluOpType.add,
                    op1=mybir.AluOpType.add,
                )
                nc.sync.dma_start(out=out_d[128 * b : 128 * (b + 1), :], in_=cum)

    nc.compile()
    return nc


_NC_CACHE = None


def kernel(x, W_hazard, b_hazard, W_base, b_base):
    global _NC_CACHE
    if _NC_CACHE is None:
        _NC_CACHE = _build_program()
    nc = _NC_CACHE

    x = np.ascontiguousarray(np.asarray(x, dtype=np.float32))
    W_cat = np.concatenate(
        [np.asarray(W_hazard, np.float32), np.asarray(W_base, np.float32)], axis=0
    )  # [513, 2048]
    bias_row = np.concatenate(
        [np.asarray(b_hazard, np.float32), np.asarray(b_base, np.float32)]
    )  # [513]
    wt = np.concatenate([W_cat.T, bias_row[None, :]], axis=0)  # [2049, 513]
    wt = np.ascontiguousarray(
        np.concatenate([wt, np.zeros((D + 1, TP - (T + 1)), np.float32)], axis=1)
    )  # [2049, 516]

    ident = np.ascontiguousarray(np.eye(128, dtype=np.float32))
    ones = np.ones((1, 128), dtype=np.float32)
    in_maps = [
        {"x": x[BLOC * i : BLOC * (i + 1)], "wt": wt, "ident": ident, "ones": ones}
        for i in range(NCORES)
    ]
    res = run_bass_kernel_spmd(nc, in_maps, list(range(NCORES)))
    return np.concatenate([res.results[i]["out"] for i in range(NCORES)], axis=0)


# revision 8
# speedup vs baseline: 1.1851x; 1.1851x over previous
"""Cumulative-probability head on 8 Trainium2 NeuronCores.

out[b, j] = sum_{i<=j} relu(x @ W_h^T + b_h)[b, i] + (x @ W_base^T + b_base)[b]

Data-parallel: x is sharded along batch (1024 rows per core); the small
weights are replicated. The host passes x pre-transposed per shard
([D, 1024], contiguous rows) so the contraction dim lands on SBUF
partitions with no on-device transposes. Per core:
  1. haz = xT.T @ WT_aug in float32r (FP22 multiplies, fp32 PSUM accum),
     PSUM-accumulated over 16 K=128 chunks. WT_aug is [2049, 516]:
     hazard cols 0..511, base col 512, 3 zero pad cols; row 2048 is the
     bias row, folded in with a K=1 ones-row matmul. The 516 output cols
     split into two even N=258 PSUM banks (fp32r requires even N).
  2. ReLU on ScalarE (hazard cols only; base col copied unactivated)
  3. inclusive cumsum along T via DVE tensor_tensor_scan with the base
     hazard as the per-partition initial state.
"""

import numpy as np

import concourse.bass as bass
import concourse.tile as tile
from concourse import bacc, mybir
from concourse.bass_utils import run_bass_kernel_spmd

B, D, T = 8192, 2048, 512
NCORES = 8
BLOC = B // NCORES            # 1024 rows per core
NB = BLOC // 128              # 8 batch tiles per core
NK = D // 128                 # 16 contraction chunks
TP = 516                      # padded output width (512 hazard + base + 3 junk)
NA = 258                      # output cols 0..257 in PSUM bank A
NBK = TP - NA                 # cols 258..515 in bank B (col 512 = base)
BOFF = T - NA                 # offset of the base col inside bank B (254)

F32 = mybir.dt.float32
F32R = mybir.dt.float32r


def _build_program():
    nc = bacc.Bacc("TRN2", target_bir_lowering=False, debug=False)

    xt_d = nc.dram_tensor("xt", [D, BLOC], F32R, kind="ExternalInput")
    wt_d = nc.dram_tensor("wt", [D + 1, TP], F32R, kind="ExternalInput")
    ones_d = nc.dram_tensor("ones", [1, 128], F32R, kind="ExternalInput")
    out_d = nc.dram_tensor("out", [BLOC, T], F32, kind="ExternalOutput")

    with tile.TileContext(nc) as tc:
        with (
            tc.tile_pool(name="consts", bufs=1) as consts,
            tc.tile_pool(name="wt", bufs=1) as wtp,
            tc.tile_pool(name="xt", bufs=1) as xtp,
            tc.tile_pool(name="haz", bufs=2) as hazp,
            tc.tile_pool(name="outp", bufs=3) as outp,
            tc.tile_pool(name="ps_mm", bufs=3, space="PSUM") as ps_mm,
        ):
            ones_r = consts.tile([1, 128], F32R)
            nc.scalar.dma_start(out=ones_r, in_=ones_d[:])
            zeros = consts.tile([128, T], F32)
            nc.vector.memset(zeros, 0.0)

            # Interleave the x^T and W^T chunk loads so the first matmuls
            # can start as soon as chunk 0 of each has landed. x^T chunks
            # issue on the Sync HWDGE queue, W^T chunks on the Scalar one.
            xt_tiles = []
            wt_tiles = []
            for k in range(NK):
                xk = xtp.tile([128, BLOC], F32R, tag=f"xt{k}")
                nc.sync.dma_start(out=xk, in_=xt_d[128 * k : 128 * (k + 1), :])
                xt_tiles.append(xk)
                w = wtp.tile([128, TP], F32R, tag=f"wt{k}")
                nc.scalar.dma_start(out=w, in_=wt_d[128 * k : 128 * (k + 1), :])
                wt_tiles.append(w)
            wbias = wtp.tile([1, TP], F32R, tag="wbias")
            nc.scalar.dma_start(out=wbias, in_=wt_d[D : D + 1, :])

            for b in range(NB):
                pA = ps_mm.tile([128, NA], F32, tag="pA")
                pB = ps_mm.tile([128, NBK], F32, tag="pB")
                for k in range(NK):
                    xt_ap = xt_tiles[k][:, 128 * b : 128 * (b + 1)]
                    w = wt_tiles[k]
                    nc.tensor.matmul(
                        pA[:],
                        xt_ap,
                        w[:, 0:NA],
                        start=(k == 0),
                        stop=False,
                    )
                    nc.tensor.matmul(
                        pB[:],
                        xt_ap,
                        w[:, NA:TP],
                        start=(k == 0),
                        stop=False,
                    )
                # Bias row: K=1 matmul against a ones row.
                nc.tensor.matmul(
                    pA[:],
                    ones_r,
                    wbias[:, 0:NA],
                    start=False,
                    stop=True,
                )
                nc.tensor.matmul(
                    pB[:],
                    ones_r,
                    wbias[:, NA:TP],
                    start=False,
                    stop=True,
                )

                haz = hazp.tile([128, T], F32, tag="haz")
                base = hazp.tile([128, 1], F32, tag="base")
                nc.scalar.activation(
                    out=haz[:, 0:NA],
                    in_=pA[:],
                    func=mybir.ActivationFunctionType.Relu,
                )
                nc.scalar.activation(
                    out=haz[:, NA:T],
                    in_=pB[:, 0:BOFF],
                    func=mybir.ActivationFunctionType.Relu,
                )
                nc.scalar.copy(out=base, in_=pB[:, BOFF : BOFF + 1])

                cum = outp.tile([128, T], F32)
                nc.vector.tensor_tensor_scan(
                    out=cum,
                    data0=haz,
                    data1=zeros,
                    initial=base,
                    op0=mybir.AluOpType.add,
                    op1=mybir.AluOpType.add,
                )
                nc.sync.dma_start(out=out_d[128 * b : 128 * (b + 1), :], in_=cum)

    nc.compile()
    return nc


_NC_CACHE = None


def kernel(x, W_hazard, b_hazard, W_base, b_base):
    global _NC_CACHE
    if _NC_CACHE is None:
        _NC_CACHE = _build_program()
    nc = _NC_CACHE

    x = np.asarray(x, dtype=np.float32)
    W_cat = np.concatenate(
        [np.asarray(W_hazard, np.float32), np.asarray(W_base, np.float32)], axis=0
    )  # [513, 2048]
    bias_row = np.concatenate(
        [np.asarray(b_hazard, np.float32), np.asarray(b_base, np.float32)]
    )  # [513]
    wt = np.concatenate([W_cat.T, bias_row[None, :]], axis=0)  # [2049, 513]
    wt = np.ascontiguousarray(
        np.concatenate([wt, np.zeros((D + 1, TP - (T + 1)), np.float32)], axis=1)
    )  # [2049, 516]

    ones = np.ones((1, 128), dtype=np.float32)
    in_maps = [
        {
            "xt": np.ascontiguousarray(x[BLOC * i : BLOC * (i + 1)].T),
            "wt": wt,
            "ones": ones,
        }
        for i in range(NCORES)
    ]
    res = run_bass_kernel_spmd(nc, in_maps, list(range(NCORES)))
    return np.concatenate([res.results[i]["out"] for i in range(NCORES)], axis=0)
